# revision 10
# baseline (speedup 1.0000x reference)
# Bass/Tile kernel for nn_LongTermAttention (continuous long-term attention
# with rectangular basis functions) on 8 Trainium2 NeuronCores.
#
# Mathematical rewrite (verified exact vs the reference):
#   * G = F^T (F F^T + ridge I)^{-1} for the rectangular basis on the padded
#     uniform grid collapses to G[l, n] = (1/4.5) * [l // 4 == n], so
#     Bc[b,n,e] = (1/4.5) * sum_{j<4} k[b,e,4n+j]  (4-wide sum pooling).
#   * psi on the integration grid is a one-hot selector, so the P=1000-point
#     continuous softmax reduces to basis space with quadrature mass Wn per
#     basis:  p_n = exp(s_n) Wn / Z,  Z = sum_n exp(s_n) Wn + w_last.
#     Wn is folded into the VALUES tiles (and the Z "ones" column), so the
#     device computes a bias-free exp.
#   * The max-subtraction in the reference cancels exactly.
#
# Performance structure (v3):
#   * k stays in original [b, e, l] layout; pooling via single-pass
#     vector.pool_avg (innermost-dim reduce, x4 folded into weight scales),
#     with gpsimd covering half-tiles via a 2-op add tree.
#   * All SBUF tiles are persistent (unique tags) so no DMA trigger ever
#     blocks on tile recycling.
#   * Scores for head-pair m are emitted right after keysT[m] drains, so the
#     scalar-engine exp stream (the critical tail) starts as early as
#     possible; exp is the ONLY scalar work.
#   * PE warm-up matmuls (zeros) ramp the tensor-engine p-state before real
#     data lands; emission order keeps the PE busy throughout.
#   * Normalize: vector does psum->sbuf cast + Z reciprocal; gpsimd does the
#     per-head scaling from SBUF.
#
# Sharding: data-parallel over batch, 2 batches per core; weights replicated.

import numpy as np

B_FULL = 16
N_CORES = 8
B_PER = B_FULL // N_CORES  # 2
E = 512          # embed dim
L = 2048         # memory length
T = 256          # query length
N = 512          # basis count
H = 8            # heads
D = 64           # head dim
P_GRID = 1000    # integration points
RIDGE_C = 4.5    # F F^T diag (4.0) + ridge (0.5)

N_WARM1 = 9      # junk matmuls before first real matmul
N_WARM2 = 4      # junk matmuls inside the first keysT chain gap

_CACHE = {}


def _host_constants(Wk, Wv):
    """Pre-scale and lay out weights; build quadrature-mass vectors.
    Scales fold: pooling 1/4.5 and query scale 1/8 into Wk; 1/4.5 into Wv
    (device pooling is a plain 4-wide SUM)."""
    import ml_dtypes
    bf16 = ml_dtypes.bfloat16
    wk = (Wk.astype(np.float64) / (RIDGE_C * 8.0)).astype(np.float32)
    wv = (Wv.astype(np.float64) / RIDGE_C).astype(np.float32)
    # layout [p, kk, e'] with e = kk*128 + p
    wk_l = np.ascontiguousarray(
        wk.reshape(4, 128, 512).transpose(1, 0, 2)).astype(bf16)
    wv_l = np.ascontiguousarray(
        wv.reshape(4, 128, 512).transpose(1, 0, 2)).astype(bf16)
    # quadrature mass per basis (trapezoid weights summed per bin, p<999)
    p = np.arange(P_GRID)
    nmap = (512 * p) // 999
    w = np.full(P_GRID, 1.0 / 999.0)
    w[0] = w[-1] = 1.0 / 1998.0
    Wn = np.zeros(N)
    for i in range(P_GRID - 1):
        Wn[nmap[i]] += w[i]
    # wn8[p, ms, h] = Wn[ms*128 + p] replicated over 8 heads
    wn8 = np.ascontiguousarray(
        np.repeat(Wn.reshape(4, 128).T[:, :, None], H, axis=2)
    ).astype(np.float32)
    w_last = float(w[-1])
    return wk_l, wv_l, wn8, w_last


def _build_program(w_last):
    import concourse.bass as bass
    import concourse.mybir as mybir
    import concourse.tile as tile
    from concourse import bacc

    nc = bacc.Bacc(
        "TRN2",
        target_bir_lowering=False,
        debug=False,
        enable_asserts=False,
        num_devices=N_CORES,
    )

    f32 = mybir.dt.float32
    bf16 = mybir.dt.bfloat16

    k_d = nc.dram_tensor("k", [B_PER, 4, 128, L], bf16,
                         kind="ExternalInput").ap()
    q_d = nc.dram_tensor("q", [B_PER, 128, 4, T], bf16,
                         kind="ExternalInput").ap()
    wk_d = nc.dram_tensor("wk", [128, 4, E], bf16, kind="ExternalInput").ap()
    wv_d = nc.dram_tensor("wv", [128, 4, E], bf16, kind="ExternalInput").ap()
    wn8_d = nc.dram_tensor("wn8", [128, 4, H], f32, kind="ExternalInput").ap()
    out_d = nc.dram_tensor("out", [B_PER, T, E], f32,
                           kind="ExternalOutput").ap()

    from contextlib import ExitStack
    with tile.TileContext(nc) as tc, ExitStack() as ctx:
        _kernel_body(ctx, tc, nc, mybir, k_d, q_d, wk_d, wv_d, wn8_d, out_d,
                     w_last)

    nc.compile()
    return nc


def _kernel_body(ctx, tc, nc, mybir, k_d, q_d, wk_d, wv_d, wn8_d, out_d,
                 w_last):
    f32 = mybir.dt.float32
    bf16 = mybir.dt.bfloat16
    Exp = mybir.ActivationFunctionType.Exp
    AVG = mybir.PoolFunctionType.avg

    def pool(name, bufs, space="SBUF"):
        return ctx.enter_context(tc.tile_pool(name=name, bufs=bufs,
                                              space=space))

    # every SBUF tile is persistent (unique tag) so nothing ever blocks on
    # tile recycling
    sb = pool("sb", 1)
    t1p = pool("t1p", 4)
    ps_proj = pool("ps_proj", 2, "PSUM")  # [128, 512] f32 (1 bank each)
    ps_sc = pool("ps_sc", 2, "PSUM")      # [128, 1024] f32 (2 banks each)
    ps_ctx = pool("ps_ctx", 2, "PSUM")    # [128, 260] f32

    def sbt(shape, dtype, tag):
        return sb.tile(shape, dtype, tag=tag, name=tag)

    # ---------------- constants / DMA priority queues ----------------
    # act ring (scalar): wn8, wk, k(b0,1), k(b0,3), wv, k(b1,1), k(b1,3),
    #                    then b0 output
    # sync ring:         k(b0,0), k(b0,2), q0, k(b1,0), k(b1,2), q1,
    #                    then b1 output
    wn8_sb = sbt([128, 4 * H], f32, "wn8")
    nc.scalar.dma_start(wn8_sb[:].rearrange("p (m h) -> p m h", m=4),
                        wn8_d[:])
    wk_sb = sbt([128, 4 * E], bf16, "wk")
    nc.scalar.dma_start(wk_sb[:].rearrange("p (kk e) -> p kk e", kk=4),
                        wk_d[:])
    wv_sb = sbt([128, 4 * E], bf16, "wv")
    qT = [sbt([128, 4 * T], bf16, f"qT{b}") for b in range(B_PER)]
    junk_sb = sbt([128, 512], bf16, "junk")
    nc.vector.memset(junk_sb[:], 0.0)

    kt_tiles = {}

    def dma_k(b, et):
        kt = sbt([128, L], bf16, f"k{b}_{et}")
        eng = nc.sync if et % 2 == 0 else nc.scalar
        eng.dma_start(kt[:], k_d[b, et])
        kt_tiles[(b, et)] = kt

    dma_k(0, 0)                      # sync
    dma_k(0, 1)                      # act
    dma_k(0, 2)                      # sync
    dma_k(0, 3)                      # act
    nc.sync.dma_start(qT[0][:].rearrange("p (e t) -> p e t", e=4), q_d[0])
    nc.scalar.dma_start(wv_sb[:].rearrange("p (kk e) -> p kk e", kk=4),
                        wv_d[:])
    dma_k(1, 0)
    dma_k(1, 1)
    dma_k(1, 2)
    dma_k(1, 3)
    nc.sync.dma_start(qT[1][:].rearrange("p (e t) -> p e t", e=4), q_d[1])

    # ---------------- values tiles + persistent Z columns ----------------
    # values[(b, ms)]: [n 128, (h, 65)] bf16; col 64 of each head block holds
    # Wn (the Z quadrature column), written once up front by gpsimd.
    values = {(b, ms): sbt([128, H * 65], bf16, f"val{b}_{ms}")
              for b in range(B_PER) for ms in range(4)}
    wn8v = wn8_sb[:].rearrange("p (m h) -> p m h", m=4)
    for b in range(B_PER):
        for ms in range(4):
            vv = values[(b, ms)][:].rearrange("p (h c) -> p h c", c=65)
            nc.gpsimd.tensor_copy(vv[:, :, 64], wn8v[:, ms, :])

    # ---------------- pooling (4-wide sum over j) ----------------
    # vector: full et0/et2 tiles + left halves of et1/et3 via single-pass
    # tensor_reduce(add); gpsimd: right halves of et1/et3 via a 2-op tree.
    pooled = {}
    AXX = mybir.AxisListType.X
    ADD = mybir.AluOpType.add

    def vreduce(pl_ap, kv_ap):
        with nc.allow_low_precision("4-wide pooling sum in bf16"):
            nc.vector.tensor_reduce(pl_ap, kv_ap, axis=AXX, op=ADD)

    def do_pool(b, et):
        kv = kt_tiles[(b, et)][:].rearrange("p (n j) -> p n j", j=4)
        pl = sbt([128, N], bf16, f"pl{b}_{et}")
        pooled[(b, et)] = pl
        if et % 2 == 0:
            vreduce(pl[:], kv[:, :, :])
        else:
            vreduce(pl[:, 0:256], kv[:, 0:256, :])
            t1 = t1p.tile([128, 512], bf16, tag="t1", name=f"t1_{b}_{et}")
            t1v = t1[:].rearrange("p (n j) -> p n j", j=2)
            nc.gpsimd.tensor_add(t1v[:, :, :], kv[:, 256:512, 0:2],
                                 kv[:, 256:512, 2:4])
            nc.gpsimd.tensor_add(pl[:, 256:512], t1v[:, :, 0], t1v[:, :, 1])

    # b0 pooling
    for et in range(4):
        do_pool(0, et)

    # ---------------- PE warm-up ----------------
    def junk_block(n):
        ps = ps_proj.tile([128, 512], f32, tag="ps_proj", name="junk")
        for _ in range(n):
            nc.tensor.matmul(ps[:], junk_sb[:, 0:128], junk_sb[:],
                             start=True, stop=True, skip_group_check=True)

    junk_block(N_WARM1)

    # ---------------- projections ----------------
    keysT = {}

    def kT_chain_mm(b, m, ps, kk, first, last):
        nc.tensor.matmul(
            ps[:],
            wk_sb[:, kk * E + m * 128: kk * E + (m + 1) * 128],
            pooled[(b, kk)][:],
            start=first, stop=last,
        )

    def kT_drain(b, m, ps):
        kt_sb = sbt([128, N], bf16, f"kT{b}_{m}")
        nc.vector.tensor_copy(kt_sb[:], ps[:])
        keysT[(b, m)] = kt_sb

    def val_chain(b, ms):
        ps = ps_proj.tile([128, 512], f32, tag="ps_proj", name=f"v{b}_{ms}")
        for i, kk in enumerate((0, 1, 2, 3)):
            nc.tensor.matmul(
                ps[:],
                pooled[(b, kk)][:, ms * 128:(ms + 1) * 128],
                wv_sb[:, kk * E:(kk + 1) * E],
                start=(i == 0), stop=(i == 3),
            )
        vv = values[(b, ms)][:].rearrange("p (h c) -> p h c", c=65)
        nc.vector.tensor_scalar_mul(
            vv[:, :, 0:64],
            ps[:].rearrange("p (h d) -> p h d", d=64),
            wn8_sb[:, ms * H: ms * H + 1],
        )

    # ---------------- scores + exp ----------------
    u_tiles = {}

    def score_block(b, hp, ab):
        """4 matmuls filling one [128,1024] psum tile (nb = 2*ab, 2*ab+1),
        then one big exp on the scalar engine."""
        ps = ps_sc.tile([128, 1024], f32, tag="ps_sc", name=f"sc{b}_{hp}_{ab}")
        for nbp in range(2):
            nb = 2 * ab + nbp
            for h01 in range(2):
                nc.tensor.matmul(
                    ps[:, h01 * 512 + nbp * 256: h01 * 512 + nbp * 256 + 256],
                    keysT[(b, hp)][h01 * 64:(h01 + 1) * 64,
                                   nb * 128:(nb + 1) * 128],
                    qT[b][h01 * 64:(h01 + 1) * 64,
                          hp * 256:(hp + 1) * 256],
                    start=True, stop=True,
                    skip_group_check=True,
                )
        u = sbt([128, 1024], bf16, f"u{b}_{hp}_{ab}")
        nc.scalar.activation(u[:], ps[:], Exp)
        u_tiles[(b, hp, ab)] = u

    # ---------------- ctx + normalize ----------------
    out_sbs = {(b, mb): sbt([128, E], f32, f"out{b}_{mb}")
               for b in range(B_PER) for mb in range(2)}

    def ctx_block(b, hp):
        """4 chains (h01, mb) x 4 nb into one [128,260] psum tile; then
        vector: rz add + recip + psum->sbuf cast; gpsimd: per-head scaling."""
        pc = ps_ctx.tile([128, 260], f32, tag="ps_ctx", name=f"ctx{b}_{hp}")
        for h01 in range(2):
            h = hp * 2 + h01
            for mb in range(2):
                c = h01 * 2 + mb
                for nb in range(4):
                    nc.tensor.matmul(
                        pc[:, c * 65:(c + 1) * 65],
                        u_tiles[(b, hp, nb // 2)][
                            :, h01 * 512 + (nb % 2) * 256 + mb * 128:
                            h01 * 512 + (nb % 2) * 256 + (mb + 1) * 128],
                        values[(b, nb)][:, h * 65: h * 65 + 65],
                        start=(nb == 0), stop=(nb == 3),
                        skip_group_check=True,
                    )
        pcv = pc[:].rearrange("p (c x) -> p c x", x=65)
        rz = sbt([128, 4], f32, f"rz{b}_{hp}")
        nc.vector.tensor_scalar_add(rz[:], pcv[:, :, 64], w_last)
        rzi = sbt([128, 4], f32, f"rzi{b}_{hp}")
        nc.vector.reciprocal(rzi[:], rz[:])
        cd = sbt([128, 260], f32, f"cd{b}_{hp}")
        nc.vector.tensor_copy(cd[:], pc[:])
        cdv = cd[:].rearrange("p (c x) -> p c x", x=65)
        for h01 in range(2):
            h = hp * 2 + h01
            for mb in range(2):
                c = h01 * 2 + mb
                nc.gpsimd.tensor_scalar_mul(
                    out_sbs[(b, mb)][:, h * 64:(h + 1) * 64],
                    cdv[:, c, 0:64], rzi[:, c:c + 1])

    # -------- b0 projections woven with early scores --------
    ps_m0 = ps_proj.tile([128, 512], f32, tag="ps_proj", name="kT0_0")
    kT_chain_mm(0, 0, ps_m0, 0, True, False)
    kT_chain_mm(0, 0, ps_m0, 1, False, False)
    junk_block(N_WARM2)
    kT_chain_mm(0, 0, ps_m0, 2, False, False)
    kT_chain_mm(0, 0, ps_m0, 3, False, True)
    kT_drain(0, 0, ps_m0)

    def kT_full(b, m):
        ps = ps_proj.tile([128, 512], f32, tag="ps_proj", name=f"kT{b}_{m}")
        for i, kk in enumerate((0, 1, 2, 3)):
            kT_chain_mm(b, m, ps, kk, i == 0, i == 3)
        kT_drain(b, m, ps)

    kT_full(0, 1)
    kT_full(0, 2)
    score_block(0, 0, 0)
    score_block(0, 0, 1)
    kT_full(0, 3)
    score_block(0, 1, 0)
    score_block(0, 1, 1)
    val_chain(0, 0)
    score_block(0, 2, 0)
    score_block(0, 2, 1)
    val_chain(0, 1)
    val_chain(0, 2)
    score_block(0, 3, 0)
    score_block(0, 3, 1)
    val_chain(0, 3)

    # b1 pooling (queued behind b0 work on vector/gpsimd)
    for et in range(4):
        do_pool(1, et)

    # -------- b1 projections woven with b0 ctx + b1 scores --------
    kT_full(1, 0)
    kT_full(1, 1)
    score_block(1, 0, 0)
    score_block(1, 0, 1)
    kT_full(1, 2)
    kT_full(1, 3)
    score_block(1, 1, 0)
    score_block(1, 1, 1)
    val_chain(1, 0)
    val_chain(1, 1)
    ctx_block(0, 0)
    score_block(1, 2, 0)
    score_block(1, 2, 1)
    val_chain(1, 2)
    val_chain(1, 3)
    ctx_block(0, 1)
    for mb in range(2):
        nc.scalar.dma_start(out_d[0, mb * 128:(mb + 1) * 128, 0:256],
                            out_sbs[(0, mb)][:, 0:256])
    score_block(1, 3, 0)
    score_block(1, 3, 1)
    ctx_block(0, 2)
    ctx_block(0, 3)
    for mb in range(2):
        nc.scalar.dma_start(out_d[0, mb * 128:(mb + 1) * 128, 256:512],
                            out_sbs[(0, mb)][:, 256:512])

    # -------- b1 ctx --------
    ctx_block(1, 0)
    ctx_block(1, 1)
    for mb in range(2):
        nc.sync.dma_start(out_d[1, mb * 128:(mb + 1) * 128, 0:256],
                          out_sbs[(1, mb)][:, 0:256])
    ctx_block(1, 2)
    ctx_block(1, 3)
    for mb in range(2):
        nc.sync.dma_start(out_d[1, mb * 128:(mb + 1) * 128, 256:512],
                          out_sbs[(1, mb)][:, 256:512])


def _get_program(w_last):
    if "nc" not in _CACHE:
        _CACHE["nc"] = _build_program(w_last)
    return _CACHE["nc"]


def make_in_maps(k, q, Wk, Wv):
    import ml_dtypes
    bf16 = ml_dtypes.bfloat16
    wk_l, wv_l, wn8, w_last = _host_constants(Wk, Wv)
    k16 = np.asarray(k).astype(bf16)
    q16 = np.asarray(q).astype(bf16)
    in_maps = []
    for c in range(N_CORES):
        ks = np.ascontiguousarray(
            k16[c * B_PER:(c + 1) * B_PER].reshape(B_PER, 4, 128, L))
        qs = q16[c * B_PER:(c + 1) * B_PER]          # [2, 256, 512]
        # -> [b, p, eb, t]
        qp = np.ascontiguousarray(
            qs.transpose(0, 2, 1).reshape(B_PER, 4, 128, T)
            .transpose(0, 2, 1, 3))
        in_maps.append({
            "k": ks,
            "q": qp,
            "wk": wk_l,
            "wv": wv_l,
            "wn8": wn8,
        })
    return in_maps, w_last


def kernel(k, q, Wk, Wv):
    from concourse.bass_utils import run_bass_kernel_spmd

    in_maps, w_last = make_in_maps(k, q, Wk, Wv)
    nc = _get_program(w_last)
    res = run_bass_kernel_spmd(nc, in_maps, core_ids=list(range(N_CORES)))
    return np.concatenate([res.results[c]["out"] for c in range(N_CORES)],
                          axis=0)


# revision 12
# speedup vs baseline: 1.3752x; 1.3752x over previous
# Bass/Tile kernel for nn_LongTermAttention (continuous long-term attention
# with rectangular basis functions) on 8 Trainium2 NeuronCores.
#
# Mathematical rewrite (verified exact vs the reference):
#   * G = F^T (F F^T + ridge I)^{-1} for the rectangular basis on the padded
#     uniform grid collapses to G[l, n] = (1/4.5) * [l // 4 == n], so
#     Bc[b,n,e] = (1/4.5) * sum_{j<4} k[b,e,4n+j]  (4-wide sum pooling).
#   * psi on the integration grid is a one-hot selector, so the P=1000-point
#     continuous softmax reduces to basis space with quadrature mass Wn per
#     basis:  p_n = exp(s_n) Wn / Z,  Z = sum_n exp(s_n) Wn + w_last.
#     Wn is folded into the VALUES tiles (and the Z "ones" column), so the
#     device computes a bias-free exp.
#   * The max-subtraction in the reference cancels exactly.
#
# Performance structure (v3):
#   * k stays in original [b, e, l] layout; pooling via single-pass
#     vector.pool_avg (innermost-dim reduce, x4 folded into weight scales),
#     with gpsimd covering half-tiles via a 2-op add tree.
#   * All SBUF tiles are persistent (unique tags) so no DMA trigger ever
#     blocks on tile recycling.
#   * Scores for head-pair m are emitted right after keysT[m] drains, so the
#     scalar-engine exp stream (the critical tail) starts as early as
#     possible; exp is the ONLY scalar work.
#   * PE warm-up matmuls (zeros) ramp the tensor-engine p-state before real
#     data lands; emission order keeps the PE busy throughout.
#   * Normalize: vector does psum->sbuf cast + Z reciprocal; gpsimd does the
#     per-head scaling from SBUF.
#
# Sharding: data-parallel over batch, 2 batches per core; weights replicated.

import numpy as np

B_FULL = 16
N_CORES = 8
B_PER = B_FULL // N_CORES  # 2
E = 512          # embed dim
L = 2048         # memory length
T = 256          # query length
N = 512          # basis count
H = 8            # heads
D = 64           # head dim
P_GRID = 1000    # integration points
RIDGE_C = 4.5    # F F^T diag (4.0) + ridge (0.5)

N_WARM1 = 10     # junk matmuls before first real matmul
N_WARM2 = 3      # junk matmuls inside the first keysT chain gap
N_WARM3 = 6      # junk matmuls while waiting for late b0 k tiles

_CACHE = {}


def _host_constants(Wk, Wv):
    """Pre-scale and lay out weights; build quadrature-mass vectors.
    Scales fold: pooling 1/4.5 and query scale 1/8 into Wk; 1/4.5 into Wv
    (device pooling is a plain 4-wide SUM)."""
    import ml_dtypes
    bf16 = ml_dtypes.bfloat16
    wk = (Wk.astype(np.float64) / (RIDGE_C * 8.0)).astype(np.float32)
    wv = (Wv.astype(np.float64) / RIDGE_C).astype(np.float32)
    # layout [p, kk, e'] with e = kk*128 + p
    wk_l = np.ascontiguousarray(
        wk.reshape(4, 128, 512).transpose(1, 0, 2)).astype(bf16)
    wv_l = np.ascontiguousarray(
        wv.reshape(4, 128, 512).transpose(1, 0, 2)).astype(bf16)
    # quadrature mass per basis (trapezoid weights summed per bin, p<999)
    p = np.arange(P_GRID)
    nmap = (512 * p) // 999
    w = np.full(P_GRID, 1.0 / 999.0)
    w[0] = w[-1] = 1.0 / 1998.0
    Wn = np.zeros(N)
    for i in range(P_GRID - 1):
        Wn[nmap[i]] += w[i]
    # wn8[p, ms, h] = Wn[ms*128 + p] replicated over 8 heads
    wn8 = np.ascontiguousarray(
        np.repeat(Wn.reshape(4, 128).T[:, :, None], H, axis=2)
    ).astype(np.float32)
    w_last = float(w[-1])
    return wk_l, wv_l, wn8, w_last


def _build_program(w_last):
    import concourse.bass as bass
    import concourse.mybir as mybir
    import concourse.tile as tile
    from concourse import bacc

    nc = bacc.Bacc(
        "TRN2",
        target_bir_lowering=False,
        debug=False,
        enable_asserts=False,
        num_devices=N_CORES,
    )

    f32 = mybir.dt.float32
    bf16 = mybir.dt.bfloat16

    k_d = nc.dram_tensor("k", [B_PER, 4, 128, L], bf16,
                         kind="ExternalInput").ap()
    q_d = nc.dram_tensor("q", [B_PER, 128, 4, T], bf16,
                         kind="ExternalInput").ap()
    wk_d = nc.dram_tensor("wk", [128, 4, E], bf16, kind="ExternalInput").ap()
    wv_d = nc.dram_tensor("wv", [128, 4, E], bf16, kind="ExternalInput").ap()
    wn8_d = nc.dram_tensor("wn8", [128, 4, H], f32, kind="ExternalInput").ap()
    out_d = nc.dram_tensor("out", [B_PER, T, E], f32,
                           kind="ExternalOutput").ap()

    from contextlib import ExitStack
    with tile.TileContext(nc) as tc, ExitStack() as ctx:
        _kernel_body(ctx, tc, nc, mybir, k_d, q_d, wk_d, wv_d, wn8_d, out_d,
                     w_last)

    nc.compile()
    return nc


def _kernel_body(ctx, tc, nc, mybir, k_d, q_d, wk_d, wv_d, wn8_d, out_d,
                 w_last):
    f32 = mybir.dt.float32
    bf16 = mybir.dt.bfloat16
    Exp = mybir.ActivationFunctionType.Exp
    AVG = mybir.PoolFunctionType.avg

    def pool(name, bufs, space="SBUF"):
        return ctx.enter_context(tc.tile_pool(name=name, bufs=bufs,
                                              space=space))

    # every SBUF tile is persistent (unique tag) so nothing ever blocks on
    # tile recycling
    sb = pool("sb", 1)
    t1p = pool("t1p", 4)
    ps_proj = pool("ps_proj", 2, "PSUM")  # [128, 512] f32 (1 bank each)
    ps_sc = pool("ps_sc", 2, "PSUM")      # [128, 1024] f32 (2 banks each)
    ps_ctx = pool("ps_ctx", 2, "PSUM")    # [128, 260] f32

    def sbt(shape, dtype, tag):
        return sb.tile(shape, dtype, tag=tag, name=tag)

    # ---------------- constants / DMA priority queues ----------------
    # act ring (scalar): wn8, wk, k(b0,1), k(b0,3), wv, k(b1,1), k(b1,3),
    #                    then b0 output
    # sync ring:         k(b0,0), k(b0,2), q0, k(b1,0), k(b1,2), q1,
    #                    then b1 output
    wn8_sb = sbt([128, 4 * H], f32, "wn8")
    nc.scalar.dma_start(wn8_sb[:].rearrange("p (m h) -> p m h", m=4),
                        wn8_d[:])
    wk_sb = sbt([128, 4 * E], bf16, "wk")
    nc.scalar.dma_start(wk_sb[:].rearrange("p (kk e) -> p kk e", kk=4),
                        wk_d[:])
    wv_sb = sbt([128, 4 * E], bf16, "wv")
    qT = [sbt([128, 4 * T], bf16, f"qT{b}") for b in range(B_PER)]
    junk_sb = sbt([128, 512], bf16, "junk")
    nc.vector.memset(junk_sb[:], 0.0)

    kt_tiles = {}

    def dma_k(b, et, eng):
        kt = sbt([128, L], bf16, f"k{b}_{et}")
        eng.dma_start(kt[:], k_d[b, et])
        kt_tiles[(b, et)] = kt

    # sync ring: b0et0, b0et2, q0, b1et0, b1et2, q1 (+ b1 out later)
    # act ring:  wn8, wk (above), b0et1, wv, b1et3 (+ b0 out later)
    # gpsimd SW-DGE ring: b0et3, b1et1
    dma_k(0, 0, nc.sync)
    dma_k(0, 1, nc.scalar)
    dma_k(0, 3, nc.gpsimd)
    dma_k(1, 1, nc.gpsimd)
    dma_k(0, 2, nc.sync)
    nc.sync.dma_start(qT[0][:].rearrange("p (e t) -> p e t", e=4), q_d[0])
    nc.scalar.dma_start(wv_sb[:].rearrange("p (kk e) -> p kk e", kk=4),
                        wv_d[:])
    nc.scalar.dma_start(qT[1][:].rearrange("p (e t) -> p e t", e=4), q_d[1])
    dma_k(1, 0, nc.sync)
    dma_k(1, 2, nc.sync)
    dma_k(1, 3, nc.scalar)

    # ---------------- values tiles + persistent Z columns ----------------
    # values[(b, ms)]: [n 128, (h, 65)] bf16; col 64 of each head block holds
    # Wn (the Z quadrature column), written once up front by gpsimd.
    values = {(b, ms): sbt([128, H * 65], bf16, f"val{b}_{ms}")
              for b in range(B_PER) for ms in range(4)}
    wn8v = wn8_sb[:].rearrange("p (m h) -> p m h", m=4)
    for b in range(B_PER):
        for ms in range(4):
            vv = values[(b, ms)][:].rearrange("p (h c) -> p h c", c=65)
            nc.gpsimd.tensor_copy(vv[:, :, 64], wn8v[:, ms, :])

    # ---------------- pooling (4-wide sum over j) ----------------
    # vector: full et0/et2 tiles + left halves of et1/et3 via single-pass
    # tensor_reduce(add); gpsimd: right halves of et1/et3 via a 2-op tree.
    pooled = {}
    AXX = mybir.AxisListType.X
    ADD = mybir.AluOpType.add

    def vreduce(pl_ap, kv_ap):
        with nc.allow_low_precision("4-wide pooling sum in bf16"):
            nc.vector.tensor_reduce(pl_ap, kv_ap, axis=AXX, op=ADD)

    def pool_v(b, et):
        kv = kt_tiles[(b, et)][:].rearrange("p (n j) -> p n j", j=4)
        pl = pooled.get((b, et))
        if pl is None:
            pl = sbt([128, N], bf16, f"pl{b}_{et}")
            pooled[(b, et)] = pl
        vreduce(pl[:, 0:256], kv[:, 0:256, :])

    def pool_g(b, et):
        kv = kt_tiles[(b, et)][:].rearrange("p (n j) -> p n j", j=4)
        pl = pooled.get((b, et))
        if pl is None:
            pl = sbt([128, N], bf16, f"pl{b}_{et}")
            pooled[(b, et)] = pl
        t1 = t1p.tile([128, 512], bf16, tag="t1", name=f"t1_{b}_{et}")
        t1v = t1[:].rearrange("p (n j) -> p n j", j=2)
        nc.gpsimd.tensor_add(t1v[:, :, :], kv[:, 256:512, 0:2],
                             kv[:, 256:512, 2:4])
        nc.gpsimd.tensor_add(pl[:, 256:512], t1v[:, :, 0], t1v[:, :, 1])

    # b0 pooling, in expected DMA-arrival order per engine
    for et in (0, 3, 2, 1):
        pool_v(0, et)
    for et in (0, 3, 2, 1):
        pool_g(0, et)

    # ---------------- PE warm-up ----------------
    def junk_block(n):
        ps = ps_proj.tile([128, 512], f32, tag="ps_proj", name="junk")
        for _ in range(n):
            nc.tensor.matmul(ps[:], junk_sb[:, 0:128], junk_sb[:],
                             start=True, stop=True, skip_group_check=True)

    junk_block(N_WARM1)

    # ---------------- projections ----------------
    keysT = {}

    def kT_chain_mm(b, m, ps, kk, first, last):
        nc.tensor.matmul(
            ps[:],
            wk_sb[:, kk * E + m * 128: kk * E + (m + 1) * 128],
            pooled[(b, kk)][:],
            start=first, stop=last,
        )

    def kT_drain(b, m, ps):
        kt_sb = sbt([128, N], bf16, f"kT{b}_{m}")
        nc.vector.tensor_copy(kt_sb[:], ps[:])
        keysT[(b, m)] = kt_sb

    def val_chain(b, ms):
        ps = ps_proj.tile([128, 512], f32, tag="ps_proj", name=f"v{b}_{ms}")
        for i, kk in enumerate((0, 1, 2, 3)):
            nc.tensor.matmul(
                ps[:],
                pooled[(b, kk)][:, ms * 128:(ms + 1) * 128],
                wv_sb[:, kk * E:(kk + 1) * E],
                start=(i == 0), stop=(i == 3),
            )
        vv = values[(b, ms)][:].rearrange("p (h c) -> p h c", c=65)
        nc.vector.tensor_scalar_mul(
            vv[:, :, 0:64],
            ps[:].rearrange("p (h d) -> p h d", d=64),
            wn8_sb[:, ms * H: ms * H + 1],
        )

    # ---------------- scores + exp ----------------
    u_tiles = {}

    def score_block(b, hp, ab):
        """4 matmuls filling one [128,1024] psum tile (nb = 2*ab, 2*ab+1),
        then one big exp on the scalar engine."""
        ps = ps_sc.tile([128, 1024], f32, tag="ps_sc", name=f"sc{b}_{hp}_{ab}")
        for nbp in range(2):
            nb = 2 * ab + nbp
            for h01 in range(2):
                nc.tensor.matmul(
                    ps[:, h01 * 512 + nbp * 256: h01 * 512 + nbp * 256 + 256],
                    keysT[(b, hp)][h01 * 64:(h01 + 1) * 64,
                                   nb * 128:(nb + 1) * 128],
                    qT[b][h01 * 64:(h01 + 1) * 64,
                          hp * 256:(hp + 1) * 256],
                    start=True, stop=True,
                    skip_group_check=True,
                )
        u = sbt([128, 1024], bf16, f"u{b}_{hp}_{ab}")
        nc.scalar.activation(u[:], ps[:], Exp)
        u_tiles[(b, hp, ab)] = u

    # ---------------- ctx + normalize ----------------
    out_sbs = {(b, mb): sbt([128, E], f32, f"out{b}_{mb}")
               for b in range(B_PER) for mb in range(2)}

    def ctx_block(b, hp):
        """4 chains (h01, mb) x 4 nb into one [128,260] psum tile; then
        vector: rz add + recip + psum->sbuf cast; gpsimd: per-head scaling."""
        pc = ps_ctx.tile([128, 260], f32, tag="ps_ctx", name=f"ctx{b}_{hp}")
        for h01 in range(2):
            h = hp * 2 + h01
            for mb in range(2):
                c = h01 * 2 + mb
                for nb in range(4):
                    nc.tensor.matmul(
                        pc[:, c * 65:(c + 1) * 65],
                        u_tiles[(b, hp, nb // 2)][
                            :, h01 * 512 + (nb % 2) * 256 + mb * 128:
                            h01 * 512 + (nb % 2) * 256 + (mb + 1) * 128],
                        values[(b, nb)][:, h * 65: h * 65 + 65],
                        start=(nb == 0), stop=(nb == 3),
                        skip_group_check=True,
                    )
        pcv = pc[:].rearrange("p (c x) -> p c x", x=65)
        rz = sbt([128, 4], f32, f"rz{b}_{hp}")
        nc.vector.tensor_scalar_add(rz[:], pcv[:, :, 64], w_last)
        rzi = sbt([128, 4], f32, f"rzi{b}_{hp}")
        nc.vector.reciprocal(rzi[:], rz[:])
        Copy = mybir.ActivationFunctionType.Copy
        for h01 in range(2):
            h = hp * 2 + h01
            for mb in range(2):
                c = h01 * 2 + mb
                dst = out_sbs[(b, mb)][:, h * 64:(h + 1) * 64]
                src = pcv[:, c, 0:64]
                if b == 0:
                    nc.vector.tensor_scalar_mul(dst, src, rzi[:, c:c + 1])
                else:
                    nc.scalar.activation(dst, src, Copy,
                                         scale=rzi[:, c:c + 1])

    # -------- b0 projections woven with early scores --------
    ps_m0 = ps_proj.tile([128, 512], f32, tag="ps_proj", name="kT0_0")
    kT_chain_mm(0, 0, ps_m0, 0, True, False)
    junk_block(N_WARM2)
    kT_chain_mm(0, 0, ps_m0, 3, False, False)
    junk_block(N_WARM3)
    kT_chain_mm(0, 0, ps_m0, 2, False, False)
    kT_chain_mm(0, 0, ps_m0, 1, False, True)
    kT_drain(0, 0, ps_m0)

    def kT_full(b, m, order=(0, 3, 2, 1)):
        ps = ps_proj.tile([128, 512], f32, tag="ps_proj", name=f"kT{b}_{m}")
        for i, kk in enumerate(order):
            kT_chain_mm(b, m, ps, kk, i == 0, i == 3)
        kT_drain(b, m, ps)

    kT_full(0, 1)
    score_block(0, 0, 0)
    score_block(0, 0, 1)
    kT_full(0, 2)
    score_block(0, 1, 0)
    score_block(0, 1, 1)
    kT_full(0, 3)
    score_block(0, 2, 0)
    score_block(0, 2, 1)
    val_chain(0, 0)
    val_chain(0, 1)
    score_block(0, 3, 0)
    score_block(0, 3, 1)
    val_chain(0, 2)
    val_chain(0, 3)

    # b1 pooling (queued behind b0 work on vector/gpsimd), arrival order:
    # b1et1 comes on the gpsimd SW ring early, b1et0/et2 on sync, et3 on act
    for et in (1, 0, 3, 2):
        pool_v(1, et)
    for et in (1, 0, 3, 2):
        pool_g(1, et)

    # -------- b1 projections woven with b0 ctx + b1 scores --------
    kT_full(1, 0, order=(1, 0, 3, 2))
    kT_full(1, 1, order=(1, 0, 3, 2))
    score_block(1, 0, 0)
    score_block(1, 0, 1)
    kT_full(1, 2, order=(1, 0, 3, 2))
    kT_full(1, 3, order=(1, 0, 3, 2))
    score_block(1, 1, 0)
    score_block(1, 1, 1)
    val_chain(1, 0)
    ctx_block(0, 0)
    score_block(1, 2, 0)
    score_block(1, 2, 1)
    val_chain(1, 1)
    val_chain(1, 2)
    ctx_block(0, 1)
    val_chain(1, 3)
    for mb in range(2):
        nc.scalar.dma_start(out_d[0, mb * 128:(mb + 1) * 128, 0:256],
                            out_sbs[(0, mb)][:, 0:256])
    score_block(1, 3, 0)
    score_block(1, 3, 1)
    ctx_block(0, 2)
    ctx_block(0, 3)
    for mb in range(2):
        nc.scalar.dma_start(out_d[0, mb * 128:(mb + 1) * 128, 256:512],
                            out_sbs[(0, mb)][:, 256:512])

    # -------- b1 ctx --------
    ctx_block(1, 0)
    ctx_block(1, 1)
    for mb in range(2):
        nc.sync.dma_start(out_d[1, mb * 128:(mb + 1) * 128, 0:256],
                          out_sbs[(1, mb)][:, 0:256])
    ctx_block(1, 2)
    ctx_block(1, 3)
    for mb in range(2):
        nc.sync.dma_start(out_d[1, mb * 128:(mb + 1) * 128, 256:512],
                          out_sbs[(1, mb)][:, 256:512])


def _get_program(w_last):
    if "nc" not in _CACHE:
        _CACHE["nc"] = _build_program(w_last)
    return _CACHE["nc"]


def make_in_maps(k, q, Wk, Wv):
    import ml_dtypes
    bf16 = ml_dtypes.bfloat16
    wk_l, wv_l, wn8, w_last = _host_constants(Wk, Wv)
    k16 = np.asarray(k).astype(bf16)
    q16 = np.asarray(q).astype(bf16)
    in_maps = []
    for c in range(N_CORES):
        ks = np.ascontiguousarray(
            k16[c * B_PER:(c + 1) * B_PER].reshape(B_PER, 4, 128, L))
        qs = q16[c * B_PER:(c + 1) * B_PER]          # [2, 256, 512]
        # -> [b, p, eb, t]
        qp = np.ascontiguousarray(
            qs.transpose(0, 2, 1).reshape(B_PER, 4, 128, T)
            .transpose(0, 2, 1, 3))
        in_maps.append({
            "k": ks,
            "q": qp,
            "wk": wk_l,
            "wv": wv_l,
            "wn8": wn8,
        })
    return in_maps, w_last


def kernel(k, q, Wk, Wv):
    from concourse.bass_utils import run_bass_kernel_spmd

    in_maps, w_last = make_in_maps(k, q, Wk, Wv)
    nc = _get_program(w_last)
    res = run_bass_kernel_spmd(nc, in_maps, core_ids=list(range(N_CORES)))
    return np.concatenate([res.results[c]["out"] for c in range(N_CORES)],
                          axis=0)
